# revision 4
# baseline (speedup 1.0000x reference)
"""Trainium2 Bass kernel for GroverMPNEncoder-style GNN message passing.

  for _ in range(6):
      agg = segment_sum(x[row], col, N) / clip(deg,1)
      x = x + relu(agg @ W.T + b)

Strategy (8 NeuronCores, one chip, ONE kernel launch for all 6 steps):
  - Destination nodes sharded contiguously across 8 cores (12500/core).
  - Host permutes nodes: within each core's range, sort dsts by degree desc,
    group into blocks of 128 dsts; block b gets K_b slots/dst (global degree
    class, identical across cores so the SPMD program is identical).
  - x lives replicated per-core in HBM as an fp8e3 (e3m4) table
    [100352, 128] in CHUNK-MAJOR permuted order: the table is the concat of
    two half-tables (blocks 0-48 of every core, then blocks 49-97), so each
    half can be AllGathered separately and the first half's collective
    overlaps the second half's compute.
  - Gather: one indirect DMA per slot column (128 dst rows / call) — the HW
    consumes exactly one dynamic index per partition per call. fp8 halves
    DMA bytes vs bf16; e3m4 keeps 4 mantissa bits (|x| < 15.5 max normal,
    final rel err ~5e-3 vs the 2e-2 gate).
  - Per block: DVE strided reduce over the K axis, ACT scale by 1/deg
    (per-partition scalar) + cast to bf16, PE transpose, PE matmul with W^T
    (+rank-1 bias matmul into PSUM), ACT relu, DVE residual add into
    persistent SBUF fp32 state x_R [128, 12544], ACT cast of updated rows
    into an fp8 SBUF stage.
  - Per step: two stage DMAs + two AllGather collectives (one per table
    half) into the next replicated table. No host round-trips.
  - Step 0's slot matrix is pre-expanded on the host (the gather indices and
    the initial x are both known), so step 0 issues cheap contiguous loads on
    the SP engine instead of 1.6k indirect DMAs on Pool.
  - AllGather in/out access patterns are expressed 1-D-major so the
    descriptor shape matches the flat blob the collective actually moves.
"""

import sys

sys.path.insert(0, "/opt/trn_rl_repo")

import numpy as np

N_NODES = 100000
N_EDGES = 1600000
HIDDEN = 128
DEPTH = 6
M = 8                      # cores
SHARD_REAL = 12500         # real dst nodes per core
SHARD = 12544              # padded shard rows (98 * 128)
NBLK = SHARD // 128        # 98 blocks per core
NTOT = M * SHARD           # 100352 rows in the replicated x table
# collective chunking: blocks are split into NCHUNK groups; each group's
# shard rows are AllGathered as soon as the group's blocks finish, so all
# but the last chunk's collective hide under later compute.
CHUNK_BLKS = [98]
CHUNK_B0 = np.concatenate([[0], np.cumsum(CHUNK_BLKS)]).astype(np.int64)
NCHUNK = len(CHUNK_BLKS)
# chunk-major global row for (core m, local row l in chunk c at offset o):
#   glob = M * (CHUNK_B0[c] * 128) + m * (CHUNK_BLKS[c] * 128) + o
_pad_c = NCHUNK - 1
ZERO_ROW = int(M * CHUNK_B0[_pad_c] * 128
               + (SHARD_REAL - CHUNK_B0[_pad_c] * 128))  # core0 row 12500


def _glob_row(m, l):
    """Chunk-major global table row for core m, local shard row l."""
    l = np.asarray(l)
    m = np.asarray(m)
    blk = l // 128
    c = np.searchsorted(CHUNK_B0[1:], blk, side="right")
    off = l - CHUNK_B0[c] * 128
    return (M * CHUNK_B0[c] * 128 + m * np.asarray(CHUNK_BLKS)[c] * 128
            + off)


def preprocess(x, edge_index, W_h_weight, W_h_bias):
    """Numpy-only host prep. Returns per-core tensors + the K_b schedule."""
    import ml_dtypes

    row = np.asarray(edge_index[0], dtype=np.int64)
    col = np.asarray(edge_index[1], dtype=np.int64)
    x = np.asarray(x, dtype=np.float32)

    deg = np.bincount(col, minlength=N_NODES).astype(np.int64)

    # permutation: node -> (core, local row); local rows degree-desc per core
    node_core = np.empty(N_NODES, dtype=np.int64)
    node_loc = np.empty(N_NODES, dtype=np.int64)
    inv_rows = np.empty(M * SHARD_REAL, dtype=np.int64)  # core-compact -> orig
    degs_sorted_per_core = []
    for m in range(M):
        lo, hi = m * SHARD_REAL, (m + 1) * SHARD_REAL
        nodes = np.arange(lo, hi)
        order = np.argsort(-deg[lo:hi], kind="stable")
        node_core[nodes[order]] = m
        node_loc[nodes[order]] = np.arange(SHARD_REAL)
        inv_rows[m * SHARD_REAL + np.arange(SHARD_REAL)] = nodes[order]
        degs_sorted_per_core.append(deg[lo:hi][order])

    # per-block slot count: exact max degree over the block across all cores
    # (identical on every core, so the SPMD program stays identical)
    K = []
    for b in range(NBLK):
        kmax = 1
        for m in range(M):
            ds = degs_sorted_per_core[m]
            if b * 128 < SHARD_REAL:
                kmax = max(kmax, int(ds[b * 128]))
        K.append(kmax)
    K_off = np.concatenate([[0], np.cumsum(K)]).astype(np.int64)
    K_total = int(K_off[-1])

    # chunk-major fp8 table + per-core transposed fp32 residual state
    x_tab = np.zeros((NTOT, HIDDEN), dtype=np.float32)
    for m in range(M):
        loc = np.arange(SHARD_REAL)
        rows = _glob_row(m, loc)
        x_tab[rows] = x[inv_rows[m * SHARD_REAL:(m + 1) * SHARD_REAL]]

    srow = _glob_row(node_core[row], node_loc[row])  # table row per edge src
    dcore = node_core[col]
    dloc = node_loc[col]

    gidx_all = []
    invdeg_all = []
    xr_all = []
    for m in range(M):
        sel = dcore == m
        ld = dloc[sel].astype(np.int64)
        sr = srow[sel].astype(np.int64)
        o = np.argsort(ld, kind="stable")
        ld, sr = ld[o], sr[o]
        # within-dst slot index
        cnt = np.bincount(ld, minlength=SHARD)
        starts = np.concatenate([[0], np.cumsum(cnt)[:-1]])
        k_in_dst = np.arange(ld.size) - starts[ld]
        b_arr = ld // 128
        d_arr = ld % 128
        cols = K_off[b_arr] + k_in_dst
        gidx = np.full((128, K_total), ZERO_ROW, dtype=np.int32)
        gidx[d_arr, cols] = sr.astype(np.int32)
        gidx_all.append(gidx)

        dg = np.zeros((128, NBLK), dtype=np.float32)
        ds = np.maximum(degs_sorted_per_core[m], 1).astype(np.float32)
        full = np.zeros(SHARD, dtype=np.float32)
        full[:SHARD_REAL] = 1.0 / ds
        dg[:, :] = full.reshape(NBLK, 128).T
        invdeg_all.append(dg)

        # x_R init: [128 part, NBLK*128] with x_R[d, b*128+f] = x(core m, local b*128+d)[f]
        xs = x[inv_rows[m * SHARD_REAL:(m + 1) * SHARD_REAL]]
        xs = np.concatenate([xs, np.zeros((SHARD - SHARD_REAL, HIDDEN),
                                          np.float32)], axis=0)
        xr = xs.reshape(NBLK, 128, HIDDEN).transpose(1, 0, 2).reshape(
            128, NBLK * HIDDEN)
        xr_all.append(np.ascontiguousarray(xr, dtype=np.float32))

    x_tab_f8 = x_tab.astype(ml_dtypes.float8_e3m4)
    # step-0 slot matrix pre-expanded on host: G0[p, j*128:(j+1)*128] =
    # x_tab_f8[gidx[p, j]]  -> [128, K_total*128] per core
    g0_all = []
    for m in range(M):
        g0 = x_tab_f8[gidx_all[m]]            # [128, K_total, 128]
        g0_all.append(np.ascontiguousarray(g0.reshape(128, K_total * 128)))
    Wt = np.ascontiguousarray(np.asarray(W_h_weight, np.float32).T).astype(
        ml_dtypes.bfloat16)
    brow = np.asarray(W_h_bias, np.float32)[None, :].astype(ml_dtypes.bfloat16)
    ident = np.eye(128, dtype=np.float32).astype(ml_dtypes.bfloat16)
    ident8 = np.eye(128, dtype=np.float32).astype(ml_dtypes.float8_e3m4)

    return dict(K=K, K_total=K_total, x_tab_f8=x_tab_f8, gidx=gidx_all,
                invdeg=invdeg_all, xr=xr_all, Wt=Wt, brow=brow, ident=ident,
                ident8=ident8, inv_rows=inv_rows, g0=g0_all)


_PROG_CACHE = {}


def build_program(K, depth=DEPTH):
    """Build + compile the SPMD Bass program for block-degree schedule K."""
    import concourse.bass as bass
    import concourse.bacc as bacc
    import concourse.mybir as mybir
    import concourse.tile as tile

    K_total = int(np.sum(K))
    K_off = np.concatenate([[0], np.cumsum(K)]).astype(np.int64)
    f32, bf16, i32 = mybir.dt.float32, mybir.dt.bfloat16, mybir.dt.int32
    f8 = mybir.dt.float8e3

    nc = bacc.Bacc("TRN2", target_bir_lowering=False, debug=False,
                   enable_asserts=False, num_devices=M)

    g0_d = nc.dram_tensor("g0", [128, K_total * HIDDEN], f8, kind="ExternalInput").ap()
    gidx_d = nc.dram_tensor("gidx", [128, K_total], i32, kind="ExternalInput").ap()
    invdeg_d = nc.dram_tensor("invdeg", [128, NBLK], f32, kind="ExternalInput").ap()
    xr_d = nc.dram_tensor("xr0", [128, NBLK * HIDDEN], f32, kind="ExternalInput").ap()
    wt_d = nc.dram_tensor("wt", [HIDDEN, HIDDEN], bf16, kind="ExternalInput").ap()
    brow_d = nc.dram_tensor("brow", [1, HIDDEN], bf16, kind="ExternalInput").ap()
    ident_d = nc.dram_tensor("ident", [128, 128], bf16, kind="ExternalInput").ap()
    ident8_d = nc.dram_tensor("ident8", [128, 128], f8, kind="ExternalInput").ap()
    out_x = nc.dram_tensor("out_x", [SHARD, HIDDEN], f32, kind="ExternalOutput").ap()

    with tile.TileContext(nc, trace_sim=False) as tc:
        with (
            tc.tile_pool(name="const", bufs=1) as cpool,
            tc.tile_pool(name="state", bufs=1) as spool,
            tc.tile_pool(name="g", bufs=3) as gpool,
            tc.tile_pool(name="work", bufs=3) as wpool,
            tc.tile_pool(name="psum", bufs=2, space="PSUM") as ppool,
            tc.tile_pool(name="dram", bufs=2, space="DRAM") as dpool,
        ):
            gidx_sb = cpool.tile([128, K_total], i32, tag="gidx")
            invdeg_sb = cpool.tile([128, NBLK], f32, tag="invdeg")
            wt_sb = cpool.tile([HIDDEN, HIDDEN], bf16, tag="wt")
            brow_sb = cpool.tile([1, HIDDEN], bf16, tag="brow")
            ident_sb = cpool.tile([128, 128], bf16, tag="ident")
            ident8_sb = cpool.tile([128, 128], f8, tag="ident8")
            ones1_sb = cpool.tile([1, 128], bf16, tag="ones1")
            oneslast_sb = cpool.tile([1, 128], bf16, tag="oneslast")
            x_R = spool.tile([128, NBLK * HIDDEN], f32, tag="xR")
            stage = spool.tile([128, NBLK * HIDDEN], f8, tag="stage")

            nc.sync.dma_start(gidx_sb[:], gidx_d[:])
            nc.sync.dma_start(invdeg_sb[:], invdeg_d[:])
            nc.sync.dma_start(wt_sb[:], wt_d[:])
            nc.sync.dma_start(brow_sb[:], brow_d[:])
            nc.sync.dma_start(ident_sb[:], ident_d[:])
            nc.sync.dma_start(ident8_sb[:], ident8_d[:])
            nc.sync.dma_start(x_R[:], xr_d[:])
            nc.vector.memset(ones1_sb[:], 1.0)
            nc.vector.memset(oneslast_sb[:], 1.0)
            if SHARD_REAL < SHARD:
                # last block: bias lands only on real dst rows
                nc.vector.memset(oneslast_sb[:, SHARD_REAL % 128:], 0.0)

            def chunk_export(xnew, xfull, c):
                """DMA stage chunk c to xnew rows + AllGather that chunk."""
                b0, nb = int(CHUNK_B0[c]), CHUNK_BLKS[c]
                bs = slice(b0 * HIDDEN, (b0 + nb) * HIDDEN)
                rs = slice(b0 * 128, (b0 + nb) * 128)
                nc.sync.dma_start(
                    xnew[rs, :].rearrange("(b d) f -> d b f", d=128),
                    stage[:, bs].rearrange("p (b f) -> p b f", f=HIDDEN))
                lo = M * b0 * 128
                n_rows = M * nb * 128
                cc = nc.gpsimd.collective_compute(
                    "AllGather",
                    mybir.AluOpType.bypass,
                    replica_groups=[list(range(M))],
                    ins=[xnew[rs, :].opt()],
                    outs=[xfull[lo:lo + n_rows, :].opt()],
                )
                # Reorder the lowered out AP dims to [[1, bytes], [1, 1]]:
                # byte-identical contiguous coverage, but the descriptor
                # shape matches the flat blob the collective moves.
                cci = cc.ins if hasattr(cc, "ins") else cc
                cci.outs[0].ap = [[1, n_rows * HIDDEN], [1, 1]]

            xsrc = None
            for t in range(depth):
                last = t == depth - 1
                if not last:
                    xnew = dpool.tile([SHARD, HIDDEN], f8, tag="xnew")
                    xfull = dpool.tile([NTOT, HIDDEN], f8, tag="xfull")
                for b in range(NBLK):
                    kb = K[b]
                    off = int(K_off[b])
                    sl = slice(b * 128, (b + 1) * 128)

                    G = gpool.tile([128, kb * 128], f8, tag="G")
                    if t == 0:
                        # step 0: host pre-expanded slot matrix, contiguous load
                        nc.sync.dma_start(
                            G[:], g0_d[:, off * 128:(off + kb) * 128])
                    else:
                        for k in range(kb):
                            nc.gpsimd.indirect_dma_start(
                                out=G[:, k * 128:(k + 1) * 128],
                                out_offset=None,
                                in_=xsrc[:],
                                in_offset=bass.IndirectOffsetOnAxis(
                                    ap=gidx_sb[:, off + k:off + k + 1], axis=0),
                            )

                    if t == 0 and kb > 1:
                        # PE identity-accumulate: psum += I^T @ G_k; keeps
                        # step 0 off the DVE (its critical path engine here)
                        pacc = ppool.tile([128, 128], f32, tag="pacc",
                                          space="PSUM")
                        for k in range(kb):
                            nc.tensor.matmul(
                                pacc[:], lhsT=ident8_sb[:],
                                rhs=G[:, k * 128:(k + 1) * 128],
                                start=(k == 0), stop=(k == kb - 1))
                        agg_in = pacc[:]
                    elif kb > 1:
                        acc = wpool.tile([128, 128], f32, tag="acc")
                        nc.vector.reduce_sum(
                            out=acc[:],
                            in_=G[:].rearrange("p (k f) -> p f k", f=128),
                            axis=mybir.AxisListType.X)
                        agg_in = acc[:]
                    else:
                        agg_in = G[:]

                    # scale by 1/deg (per-partition scalar) + cast to bf16
                    aggs = wpool.tile([128, 128], bf16, tag="aggs")
                    nc.scalar.activation(
                        aggs[:], agg_in,
                        mybir.ActivationFunctionType.Copy,
                        scale=invdeg_sb[:, b:b + 1])

                    psumT = ppool.tile([128, 128], bf16, tag="psumT",
                                       space="PSUM")
                    nc.tensor.transpose(out=psumT[:], in_=aggs[:],
                                        identity=ident_sb[:])
                    aggT = wpool.tile([128, 128], bf16, tag="aggT")
                    nc.scalar.activation(aggT[:], psumT[:],
                                         mybir.ActivationFunctionType.Copy)

                    psumM = ppool.tile([128, 128], f32, tag="psumM",
                                       space="PSUM")
                    nc.tensor.matmul(psumM[:], lhsT=aggT[:], rhs=wt_sb[:],
                                     start=True, stop=False)
                    ones_vec = oneslast_sb if b == NBLK - 1 else ones1_sb
                    nc.tensor.matmul(psumM[:], lhsT=ones_vec[:],
                                     rhs=brow_sb[:], start=False, stop=True)

                    msg = wpool.tile([128, 128], f32, tag="msg")
                    nc.scalar.activation(msg[:], psumM[:],
                                         mybir.ActivationFunctionType.Relu)

                    nc.vector.tensor_add(x_R[:, sl], x_R[:, sl], msg[:])

                    if not last:
                        nc.scalar.activation(
                            stage[:, sl], x_R[:, sl],
                            mybir.ActivationFunctionType.Copy)
                        # emit each chunk's export+collective a few blocks
                        # after the chunk completes: by then the compute
                        # pipeline for it has drained, so the collective's
                        # input wait doesn't stall Pool's gather desc-gen
                        for c in range(NCHUNK):
                            trig = int(CHUNK_B0[c + 1]) - 1
                            if c < NCHUNK - 1:
                                trig = min(trig + 6, NBLK - 1)
                            if b == trig:
                                chunk_export(xnew, xfull, c)

                if last:
                    # out_x[b*128+d, f] = x_R[d, b*128+f]
                    nc.sync.dma_start(
                        out_x.rearrange("(b d) f -> d b f", d=128),
                        x_R[:].rearrange("p (b f) -> p b f", f=HIDDEN))
                else:
                    xsrc = xfull

    nc.compile()
    return nc


def _get_prog(K, depth=DEPTH):
    key = (tuple(K), depth)
    if key not in _PROG_CACHE:
        _PROG_CACHE[key] = build_program(list(K), depth)
    return _PROG_CACHE[key]


def _in_maps(prep):
    maps = []
    for m in range(M):
        maps.append({
            "g0": prep["g0"][m],
            "gidx": prep["gidx"][m],
            "invdeg": prep["invdeg"][m],
            "xr0": prep["xr"][m],
            "wt": np.asarray(prep["Wt"]),
            "brow": np.asarray(prep["brow"]),
            "ident": np.asarray(prep["ident"]),
            "ident8": np.asarray(prep["ident8"]),
        })
    return maps


def run_on_hw(nc, in_maps, trace=False):
    from concourse.bass_interp import get_hw_module
    from concourse.bass_utils import run_bass_kernel_spmd
    old = nc.m
    nc.m = get_hw_module(nc.m)
    try:
        return run_bass_kernel_spmd(nc, in_maps, core_ids=list(range(M)),
                                    trace=trace)
    finally:
        nc.m = old


def kernel(x, edge_index, W_h_weight, W_h_bias, _trace=False, _res_out=None):
    prep = preprocess(x, edge_index, W_h_weight, W_h_bias)
    nc = _get_prog(prep["K"])

    res = run_on_hw(nc, _in_maps(prep), trace=_trace)
    if _res_out is not None:
        _res_out.append(res)
    shards = [res.results[m]["out_x"] for m in range(M)]  # [SHARD,H] f32

    out = np.empty((N_NODES, HIDDEN), dtype=np.float32)
    for m in range(M):
        out[prep["inv_rows"][m * SHARD_REAL:(m + 1) * SHARD_REAL]] = \
            shards[m][:SHARD_REAL]
    return out



# revision 5
# speedup vs baseline: 1.2742x; 1.2742x over previous
"""Trainium2 Bass kernel for GroverMPNEncoder-style GNN message passing.

  for _ in range(6):
      agg = segment_sum(x[row], col, N) / clip(deg,1)
      x = x + relu(agg @ W.T + b)

Strategy (8 NeuronCores, one chip, ONE kernel launch for all 6 steps):
  - Destination nodes sharded contiguously across 8 cores (12500/core).
  - Host permutes nodes: within each core's range, sort dsts by degree desc,
    group into blocks of 128 dsts; block b gets K_b slots/dst (global degree
    class, identical across cores so the SPMD program is identical).
  - x lives replicated per-core in HBM as an fp8e3 (e3m4) table
    [100352, 128] in CHUNK-MAJOR permuted order: the table is the concat of
    two half-tables (blocks 0-48 of every core, then blocks 49-97), so each
    half can be AllGathered separately and the first half's collective
    overlaps the second half's compute.
  - Gather: one indirect DMA per slot column (128 dst rows / call) — the HW
    consumes exactly one dynamic index per partition per call. fp8 halves
    DMA bytes vs bf16; e3m4 keeps 4 mantissa bits (|x| < 15.5 max normal,
    final rel err ~5e-3 vs the 2e-2 gate).
  - Per block: DVE strided reduce over the K axis, ACT scale by 1/deg
    (per-partition scalar) + cast to bf16, PE transpose, PE matmul with W^T
    (+rank-1 bias matmul into PSUM), ACT relu, DVE residual add into
    persistent SBUF fp32 state x_R [128, 12544], ACT cast of updated rows
    into an fp8 SBUF stage.
  - Per step: two stage DMAs + two AllGather collectives (one per table
    half) into the next replicated table. No host round-trips.
  - Step 0's slot matrix is pre-expanded on the host (the gather indices and
    the initial x are both known), so step 0 issues cheap contiguous loads on
    the SP engine instead of 1.6k indirect DMAs on Pool.
  - AllGather in/out access patterns are expressed 1-D-major so the
    descriptor shape matches the flat blob the collective actually moves.
"""

import sys

sys.path.insert(0, "/opt/trn_rl_repo")

import numpy as np

N_NODES = 100000
N_EDGES = 1600000
HIDDEN = 128
DEPTH = 6
M = 8                      # cores
SHARD_REAL = 12500         # real dst nodes per core
SHARD = 12544              # padded shard rows (98 * 128)
NBLK = SHARD // 128        # 98 blocks per core
NTOT = M * SHARD           # 100352 rows in the replicated x table
# collective chunking: blocks are split into NCHUNK groups; each group's
# shard rows are AllGathered as soon as the group's blocks finish, so all
# but the last chunk's collective hide under later compute.
CHUNK_BLKS = [98]
CHUNK_B0 = np.concatenate([[0], np.cumsum(CHUNK_BLKS)]).astype(np.int64)
NCHUNK = len(CHUNK_BLKS)
# chunk-major global row for (core m, local row l in chunk c at offset o):
#   glob = M * (CHUNK_B0[c] * 128) + m * (CHUNK_BLKS[c] * 128) + o
_pad_c = NCHUNK - 1
ZERO_ROW = int(M * CHUNK_B0[_pad_c] * 128
               + (SHARD_REAL - CHUNK_B0[_pad_c] * 128))  # core0 row 12500


def _glob_row(m, l):
    """Chunk-major global table row for core m, local shard row l."""
    l = np.asarray(l)
    m = np.asarray(m)
    blk = l // 128
    c = np.searchsorted(CHUNK_B0[1:], blk, side="right")
    off = l - CHUNK_B0[c] * 128
    return (M * CHUNK_B0[c] * 128 + m * np.asarray(CHUNK_BLKS)[c] * 128
            + off)


def preprocess(x, edge_index, W_h_weight, W_h_bias):
    """Numpy-only host prep. Returns per-core tensors + the K_b schedule."""
    import ml_dtypes

    row = np.asarray(edge_index[0], dtype=np.int64)
    col = np.asarray(edge_index[1], dtype=np.int64)
    x = np.asarray(x, dtype=np.float32)

    deg = np.bincount(col, minlength=N_NODES).astype(np.int64)

    # permutation: node -> (core, local row); local rows degree-desc per core
    node_core = np.empty(N_NODES, dtype=np.int64)
    node_loc = np.empty(N_NODES, dtype=np.int64)
    inv_rows = np.empty(M * SHARD_REAL, dtype=np.int64)  # core-compact -> orig
    degs_sorted_per_core = []
    for m in range(M):
        lo, hi = m * SHARD_REAL, (m + 1) * SHARD_REAL
        nodes = np.arange(lo, hi)
        order = np.argsort(-deg[lo:hi], kind="stable")
        node_core[nodes[order]] = m
        node_loc[nodes[order]] = np.arange(SHARD_REAL)
        inv_rows[m * SHARD_REAL + np.arange(SHARD_REAL)] = nodes[order]
        degs_sorted_per_core.append(deg[lo:hi][order])

    # per-block slot count: exact max degree over the block across all cores
    # (identical on every core, so the SPMD program stays identical)
    K = []
    for b in range(NBLK):
        kmax = 1
        for m in range(M):
            ds = degs_sorted_per_core[m]
            if b * 128 < SHARD_REAL:
                kmax = max(kmax, int(ds[b * 128]))
        K.append(kmax)
    K_off = np.concatenate([[0], np.cumsum(K)]).astype(np.int64)
    K_total = int(K_off[-1])

    # chunk-major fp8 table + per-core transposed fp32 residual state
    x_tab = np.zeros((NTOT, HIDDEN), dtype=np.float32)
    for m in range(M):
        loc = np.arange(SHARD_REAL)
        rows = _glob_row(m, loc)
        x_tab[rows] = x[inv_rows[m * SHARD_REAL:(m + 1) * SHARD_REAL]]

    srow = _glob_row(node_core[row], node_loc[row])  # table row per edge src
    dcore = node_core[col]
    dloc = node_loc[col]

    gidx_all = []
    invdeg_all = []
    xr_all = []
    for m in range(M):
        sel = dcore == m
        ld = dloc[sel].astype(np.int64)
        sr = srow[sel].astype(np.int64)
        o = np.argsort(ld, kind="stable")
        ld, sr = ld[o], sr[o]
        # within-dst slot index
        cnt = np.bincount(ld, minlength=SHARD)
        starts = np.concatenate([[0], np.cumsum(cnt)[:-1]])
        k_in_dst = np.arange(ld.size) - starts[ld]
        b_arr = ld // 128
        d_arr = ld % 128
        cols = K_off[b_arr] + k_in_dst
        gidx = np.full((128, K_total), ZERO_ROW, dtype=np.int32)
        gidx[d_arr, cols] = sr.astype(np.int32)
        gidx_all.append(gidx)

        dg = np.zeros((128, NBLK), dtype=np.float32)
        ds = np.maximum(degs_sorted_per_core[m], 1).astype(np.float32)
        full = np.zeros(SHARD, dtype=np.float32)
        full[:SHARD_REAL] = 1.0 / ds
        dg[:, :] = full.reshape(NBLK, 128).T
        invdeg_all.append(dg)

        # x_R init: [128 part, NBLK*128] with x_R[d, b*128+f] = x(core m, local b*128+d)[f]
        xs = x[inv_rows[m * SHARD_REAL:(m + 1) * SHARD_REAL]]
        xs = np.concatenate([xs, np.zeros((SHARD - SHARD_REAL, HIDDEN),
                                          np.float32)], axis=0)
        xr = xs.reshape(NBLK, 128, HIDDEN).transpose(1, 0, 2).reshape(
            128, NBLK * HIDDEN)
        xr_all.append(np.ascontiguousarray(xr, dtype=np.float32))

    x_tab_f8 = x_tab.astype(ml_dtypes.float8_e3m4)
    # step-0 slot matrix pre-expanded on host: G0[p, j*128:(j+1)*128] =
    # x_tab_f8[gidx[p, j]]  -> [128, K_total*128] per core
    g0_all = []
    for m in range(M):
        g0 = x_tab_f8[gidx_all[m]]            # [128, K_total, 128]
        g0_all.append(np.ascontiguousarray(g0.reshape(128, K_total * 128)))
    Wt = np.ascontiguousarray(np.asarray(W_h_weight, np.float32).T).astype(
        ml_dtypes.bfloat16)
    brow = np.asarray(W_h_bias, np.float32)[None, :].astype(ml_dtypes.bfloat16)
    ident = np.eye(128, dtype=np.float32).astype(ml_dtypes.bfloat16)
    ident8 = np.eye(128, dtype=np.float32).astype(ml_dtypes.float8_e3m4)

    return dict(K=K, K_total=K_total, x_tab_f8=x_tab_f8, gidx=gidx_all,
                invdeg=invdeg_all, xr=xr_all, Wt=Wt, brow=brow, ident=ident,
                ident8=ident8, inv_rows=inv_rows, g0=g0_all)


_PROG_CACHE = {}


def build_program(K, depth=DEPTH):
    """Build + compile the SPMD Bass program for block-degree schedule K."""
    import concourse.bass as bass
    import concourse.bacc as bacc
    import concourse.mybir as mybir
    import concourse.tile as tile

    K_total = int(np.sum(K))
    K_off = np.concatenate([[0], np.cumsum(K)]).astype(np.int64)
    f32, bf16, i32 = mybir.dt.float32, mybir.dt.bfloat16, mybir.dt.int32
    f8 = mybir.dt.float8e3

    nc = bacc.Bacc("TRN2", target_bir_lowering=False, debug=False,
                   enable_asserts=False, num_devices=M)

    g0_d = nc.dram_tensor("g0", [128, K_total * HIDDEN], f8, kind="ExternalInput").ap()
    gidx_d = nc.dram_tensor("gidx", [128, K_total], i32, kind="ExternalInput").ap()
    invdeg_d = nc.dram_tensor("invdeg", [128, NBLK], f32, kind="ExternalInput").ap()
    xr_d = nc.dram_tensor("xr0", [128, NBLK * HIDDEN], f32, kind="ExternalInput").ap()
    wt_d = nc.dram_tensor("wt", [HIDDEN, HIDDEN], bf16, kind="ExternalInput").ap()
    brow_d = nc.dram_tensor("brow", [1, HIDDEN], bf16, kind="ExternalInput").ap()
    ident_d = nc.dram_tensor("ident", [128, 128], bf16, kind="ExternalInput").ap()
    ident8_d = nc.dram_tensor("ident8", [128, 128], f8, kind="ExternalInput").ap()
    out_x = nc.dram_tensor("out_x", [SHARD, HIDDEN], f32, kind="ExternalOutput").ap()

    with tile.TileContext(nc, trace_sim=False) as tc:
        with (
            tc.tile_pool(name="const", bufs=1) as cpool,
            tc.tile_pool(name="state", bufs=1) as spool,
            tc.tile_pool(name="g", bufs=3) as gpool,
            tc.tile_pool(name="work", bufs=3) as wpool,
            tc.tile_pool(name="psum", bufs=2, space="PSUM") as ppool,
            tc.tile_pool(name="dram", bufs=2, space="DRAM") as dpool,
        ):
            gidx_sb = cpool.tile([128, K_total], i32, tag="gidx")
            invdeg_sb = cpool.tile([128, NBLK], f32, tag="invdeg")
            wt_sb = cpool.tile([HIDDEN, HIDDEN], bf16, tag="wt")
            brow_sb = cpool.tile([1, HIDDEN], bf16, tag="brow")
            ident_sb = cpool.tile([128, 128], bf16, tag="ident")
            ident8_sb = cpool.tile([128, 128], f8, tag="ident8")
            ones1_sb = cpool.tile([1, 128], bf16, tag="ones1")
            oneslast_sb = cpool.tile([1, 128], bf16, tag="oneslast")
            x_R = spool.tile([128, NBLK * HIDDEN], f32, tag="xR")
            stage = spool.tile([128, NBLK * HIDDEN], f8, tag="stage")

            nc.sync.dma_start(gidx_sb[:], gidx_d[:])
            nc.sync.dma_start(invdeg_sb[:], invdeg_d[:])
            nc.sync.dma_start(wt_sb[:], wt_d[:])
            nc.sync.dma_start(brow_sb[:], brow_d[:])
            nc.sync.dma_start(ident_sb[:], ident_d[:])
            nc.sync.dma_start(ident8_sb[:], ident8_d[:])
            nc.sync.dma_start(x_R[:], xr_d[:])
            nc.vector.memset(ones1_sb[:], 1.0)
            nc.vector.memset(oneslast_sb[:], 1.0)
            if SHARD_REAL < SHARD:
                # last block: bias lands only on real dst rows
                nc.vector.memset(oneslast_sb[:, SHARD_REAL % 128:], 0.0)

            def chunk_export(xnew, xfull, c):
                """DMA stage chunk c to xnew rows + AllGather that chunk."""
                b0, nb = int(CHUNK_B0[c]), CHUNK_BLKS[c]
                bs = slice(b0 * HIDDEN, (b0 + nb) * HIDDEN)
                rs = slice(b0 * 128, (b0 + nb) * 128)
                nc.sync.dma_start(
                    xnew[rs, :].rearrange("(b d) f -> d b f", d=128),
                    stage[:, bs].rearrange("p (b f) -> p b f", f=HIDDEN))
                lo = M * b0 * 128
                n_rows = M * nb * 128
                nc.gpsimd.collective_compute(
                    "AllGather",
                    mybir.AluOpType.bypass,
                    replica_groups=[list(range(M))],
                    ins=[xnew[rs, :].opt()],
                    outs=[xfull[lo:lo + n_rows, :].opt()],
                )

            xsrc = None
            for t in range(depth):
                last = t == depth - 1
                if not last:
                    xnew = dpool.tile([SHARD, HIDDEN], f8, tag="xnew")
                    xfull = dpool.tile([NTOT, HIDDEN], f8, tag="xfull")
                for b in range(NBLK):
                    kb = K[b]
                    off = int(K_off[b])
                    sl = slice(b * 128, (b + 1) * 128)

                    G = gpool.tile([128, kb * 128], f8, tag="G")
                    if t == 0:
                        # step 0: host pre-expanded slot matrix, contiguous load
                        nc.sync.dma_start(
                            G[:], g0_d[:, off * 128:(off + kb) * 128])
                    else:
                        for k in range(kb):
                            nc.gpsimd.indirect_dma_start(
                                out=G[:, k * 128:(k + 1) * 128],
                                out_offset=None,
                                in_=xsrc[:],
                                in_offset=bass.IndirectOffsetOnAxis(
                                    ap=gidx_sb[:, off + k:off + k + 1], axis=0),
                            )

                    if t == 0 and kb > 1:
                        # PE identity-accumulate: psum += I^T @ G_k; keeps
                        # step 0 off the DVE (its critical path engine here)
                        pacc = ppool.tile([128, 128], f32, tag="pacc",
                                          space="PSUM")
                        for k in range(kb):
                            nc.tensor.matmul(
                                pacc[:], lhsT=ident8_sb[:],
                                rhs=G[:, k * 128:(k + 1) * 128],
                                start=(k == 0), stop=(k == kb - 1))
                        agg_in = pacc[:]
                    elif kb > 1:
                        acc = wpool.tile([128, 128], f32, tag="acc")
                        nc.vector.reduce_sum(
                            out=acc[:],
                            in_=G[:].rearrange("p (k f) -> p f k", f=128),
                            axis=mybir.AxisListType.X)
                        agg_in = acc[:]
                    else:
                        agg_in = G[:]

                    # scale by 1/deg (per-partition scalar) + cast to bf16
                    aggs = wpool.tile([128, 128], bf16, tag="aggs")
                    nc.scalar.activation(
                        aggs[:], agg_in,
                        mybir.ActivationFunctionType.Copy,
                        scale=invdeg_sb[:, b:b + 1])

                    psumT = ppool.tile([128, 128], bf16, tag="psumT",
                                       space="PSUM")
                    nc.tensor.transpose(out=psumT[:], in_=aggs[:],
                                        identity=ident_sb[:])
                    aggT = wpool.tile([128, 128], bf16, tag="aggT")
                    nc.scalar.activation(aggT[:], psumT[:],
                                         mybir.ActivationFunctionType.Copy)

                    psumM = ppool.tile([128, 128], f32, tag="psumM",
                                       space="PSUM")
                    nc.tensor.matmul(psumM[:], lhsT=aggT[:], rhs=wt_sb[:],
                                     start=True, stop=False)
                    ones_vec = oneslast_sb if b == NBLK - 1 else ones1_sb
                    nc.tensor.matmul(psumM[:], lhsT=ones_vec[:],
                                     rhs=brow_sb[:], start=False, stop=True)

                    msg = wpool.tile([128, 128], f32, tag="msg")
                    nc.scalar.activation(msg[:], psumM[:],
                                         mybir.ActivationFunctionType.Relu)

                    nc.vector.tensor_add(x_R[:, sl], x_R[:, sl], msg[:])

                    if not last:
                        nc.scalar.activation(
                            stage[:, sl], x_R[:, sl],
                            mybir.ActivationFunctionType.Copy)
                        # emit each chunk's export+collective a few blocks
                        # after the chunk completes: by then the compute
                        # pipeline for it has drained, so the collective's
                        # input wait doesn't stall Pool's gather desc-gen
                        for c in range(NCHUNK):
                            trig = int(CHUNK_B0[c + 1]) - 1
                            if c < NCHUNK - 1:
                                trig = min(trig + 6, NBLK - 1)
                            if b == trig:
                                chunk_export(xnew, xfull, c)

                if last:
                    # out_x[b*128+d, f] = x_R[d, b*128+f]
                    nc.sync.dma_start(
                        out_x.rearrange("(b d) f -> d b f", d=128),
                        x_R[:].rearrange("p (b f) -> p b f", f=HIDDEN))
                else:
                    xsrc = xfull

    nc.compile()
    # Reorder each collective's lowered out AP to [[1, total], [1, 1]]:
    # byte-identical contiguous coverage of the same blob, expressed with
    # the bulk extent leading. Must run post-compile (compile canonicalizes
    # APs back to a unit-leading form).
    for fn in nc.m.functions:
        for blk in fn.blocks:
            for inst in blk.instructions:
                if type(inst).__name__ == "InstCollectiveCompute":
                    tot = 1
                    for _s, _c in inst.outs[0].ap:
                        tot *= _c
                    inst.outs[0].ap = [[1, tot], [1, 1]]
    return nc


def _get_prog(K, depth=DEPTH):
    key = (tuple(K), depth)
    if key not in _PROG_CACHE:
        _PROG_CACHE[key] = build_program(list(K), depth)
    return _PROG_CACHE[key]


def _in_maps(prep):
    maps = []
    for m in range(M):
        maps.append({
            "g0": prep["g0"][m],
            "gidx": prep["gidx"][m],
            "invdeg": prep["invdeg"][m],
            "xr0": prep["xr"][m],
            "wt": np.asarray(prep["Wt"]),
            "brow": np.asarray(prep["brow"]),
            "ident": np.asarray(prep["ident"]),
            "ident8": np.asarray(prep["ident8"]),
        })
    return maps


def run_on_hw(nc, in_maps, trace=False):
    from concourse.bass_interp import get_hw_module
    from concourse.bass_utils import run_bass_kernel_spmd
    old = nc.m
    nc.m = get_hw_module(nc.m)
    try:
        return run_bass_kernel_spmd(nc, in_maps, core_ids=list(range(M)),
                                    trace=trace)
    finally:
        nc.m = old


def kernel(x, edge_index, W_h_weight, W_h_bias, _trace=False, _res_out=None):
    prep = preprocess(x, edge_index, W_h_weight, W_h_bias)
    nc = _get_prog(prep["K"])

    res = run_on_hw(nc, _in_maps(prep), trace=_trace)
    if _res_out is not None:
        _res_out.append(res)
    shards = [res.results[m]["out_x"] for m in range(M)]  # [SHARD,H] f32

    out = np.empty((N_NODES, HIDDEN), dtype=np.float32)
    for m in range(M):
        out[prep["inv_rows"][m * SHARD_REAL:(m + 1) * SHARD_REAL]] = \
            shards[m][:SHARD_REAL]
    return out

